# revision 3
# baseline (speedup 1.0000x reference)
"""RNN-T joint network kernel for 8 Trainium2 NeuronCores.

out[b,t,u,c] = (enc[b,t,:] @ W[:, :D].T)[c] + (dec[b,u,:] @ W[:, D:].T)[c]

Sharding: data-parallel over (b, t-half): core i -> b = i//2, t-slab
[(i%2)*128, (i%2+1)*128).  Each core holds the full W, computes its
(128, 64, 1024) output slab (32 MB) and DMAs it out.

Per-core dataflow:
  1. DMA in enc slab (128,512), dec slab (64,512), W (1024,1024).
  2. PE-transpose W, enc, dec so the contraction dim D sits on partitions.
  3. Two small GEMMs -> enc_proj (128,1024), dec_proj (64,1024) in SBUF.
  4. For each u: a K=64 "selector" matmul broadcasts dec_proj[u,:] across
     all 128 partitions into PSUM; DVE adds enc_proj; groups of 4 u's
     form one contiguous 2 MB DMA to DRAM.
"""

import sys

import numpy as np

for _p in ("/opt/trn_rl_repo",):
    if _p not in sys.path:
        sys.path.insert(0, _p)

B, T, U, D, C = 4, 256, 64, 512, 1024
TSH = T // 2  # t-slab per core
NCORES = 8
UG = 4  # u's per output tile / DMA (4 * 512KB = 2MB per dma_start)

_CACHE = {}


def _build_bass():
    import concourse.mybir as mybir
    from concourse import bacc
    from concourse.bass import ds
    from concourse.masks import make_identity
    from concourse.tile import TileContext

    f32 = mybir.dt.float32
    add = mybir.AluOpType.add

    nc = bacc.Bacc("TRN2", target_bir_lowering=False, debug=False)
    enc_d = nc.declare_dram_parameter("enc", [TSH, D], f32, isOutput=False)
    dec_d = nc.declare_dram_parameter("dec", [U, D], f32, isOutput=False)
    w_d = nc.declare_dram_parameter("w", [C, 2 * D], f32, isOutput=False)
    o_d = nc.declare_dram_parameter("o", [TSH, U, C], f32, isOutput=True)

    with TileContext(nc) as tc:
        with (
            tc.tile_pool(name="const", bufs=1) as cpool,
            tc.tile_pool(name="psum", bufs=2, space="PSUM") as ppool,
            tc.tile_pool(name="outp", bufs=3) as opool,
        ):
            ident = cpool.tile([128, 128], f32)
            make_identity(nc, ident[:])

            # sel[k, u, m] = 1.0 if k == u else 0.0   (k on partitions)
            sel = cpool.tile([U, U, 128], f32)
            nc.gpsimd.memset(sel[:], 0.0)
            nc.gpsimd.affine_select(
                out=sel[:],
                in_=sel[:],
                compare_op=mybir.AluOpType.not_equal,
                fill=1.0,
                base=0,
                pattern=[[-1, U], [0, 128]],
                channel_multiplier=1,
            )

            # ---- loads ----
            w_sb = cpool.tile([128, 8, 1024], f32)  # w_sb[p, ct, d] = W[ct*128+p, d]
            nc.sync.dma_start(out=w_sb[:], in_=w_d.rearrange("(ct p) d -> p ct d", p=128))
            enc_sb = cpool.tile([TSH, D], f32)
            nc.sync.dma_start(out=enc_sb[:], in_=enc_d[:])
            dec_sb = cpool.tile([U, D], f32)
            nc.sync.dma_start(out=dec_sb[:], in_=dec_d[:])

            # ---- transposes (PE) ----
            # wT[p, dt, c] = W[c, dt*128+p]  (d on partitions)
            wT = cpool.tile([128, 8, 1024], f32)
            for dt in range(8):
                for cg in range(2):
                    pt = ppool.tile([128, 4, 128], f32, tag="tp")
                    for j in range(4):
                        ct = cg * 4 + j
                        nc.tensor.transpose(
                            pt[:, j], w_sb[:, ct, ds(dt * 128, 128)], ident[:]
                        )
                    nc.any.tensor_copy(out=wT[:, dt, ds(cg * 512, 512)], in_=pt[:])

            # encT[p, dt, t] = enc[t, dt*128+p]
            encT = cpool.tile([128, 4, TSH], f32)
            pt = ppool.tile([128, 4, 128], f32, tag="tp")
            for dt in range(4):
                nc.tensor.transpose(pt[:, dt], enc_sb[:, ds(dt * 128, 128)], ident[:])
            nc.any.tensor_copy(out=encT[:], in_=pt[:])

            # decT[p, dt, u] = dec[u, dt*128+p]
            decT = cpool.tile([128, 4, U], f32)
            pt = ppool.tile([128, 4, 128], f32, tag="tp")
            for dt in range(4):
                nc.tensor.transpose(
                    pt[:, dt, :U], dec_sb[:, ds(dt * 128, 128)], ident[:U, :U]
                )
            nc.any.tensor_copy(out=decT[:], in_=pt[:, :, :U])

            # ---- projections ----
            enc_proj = cpool.tile([TSH, C], f32)
            for h in range(2):
                pp = ppool.tile([TSH, 512], f32, tag="proj")
                for dt in range(4):
                    nc.tensor.matmul(
                        pp[:],
                        encT[:, dt, :],
                        wT[:, dt, ds(h * 512, 512)],
                        start=(dt == 0),
                        stop=(dt == 3),
                    )
                nc.any.tensor_copy(out=enc_proj[:, ds(h * 512, 512)], in_=pp[:])

            dec_proj = cpool.tile([U, C], f32)
            for h in range(2):
                pp = ppool.tile([TSH, 512], f32, tag="proj")
                for dt in range(4):
                    nc.tensor.matmul(
                        pp[:U],
                        decT[:, dt, :],
                        wT[:, 4 + dt, ds(h * 512, 512)],
                        start=(dt == 0),
                        stop=(dt == 3),
                    )
                nc.any.tensor_copy(out=dec_proj[:, ds(h * 512, 512)], in_=pp[:U])

            # ---- main loop over u ----
            for ug in range(U // UG):
                ot = opool.tile([TSH, UG, C], f32, tag="out")
                for j in range(UG):
                    u = ug * UG + j
                    pr = ppool.tile([TSH, C], f32, tag="rep")
                    for h in range(2):
                        nc.tensor.matmul(
                            pr[:, ds(h * 512, 512)],
                            sel[:, u, :],
                            dec_proj[:, ds(h * 512, 512)],
                            start=True,
                            stop=True,
                        )
                    nc.vector.tensor_tensor(
                        out=ot[:, j, :], in0=pr[:], in1=enc_proj[:], op=add
                    )
                nc.sync.dma_start(out=o_d[:, ds(ug * UG, UG), :], in_=ot[:])

    nc.compile()
    return nc


def _get_nc():
    if "nc" not in _CACHE:
        _CACHE["nc"] = _build_bass()
    return _CACHE["nc"]


def _make_in_maps(encoder_outputs, decoder_outputs, W):
    enc = np.ascontiguousarray(np.asarray(encoder_outputs, dtype=np.float32))
    dec = np.ascontiguousarray(np.asarray(decoder_outputs, dtype=np.float32))
    w = np.ascontiguousarray(np.asarray(W, dtype=np.float32))
    in_maps = []
    for i in range(NCORES):
        b, th = i // 2, i % 2
        in_maps.append(
            {
                "enc": np.ascontiguousarray(enc[b, th * TSH : (th + 1) * TSH]),
                "dec": np.ascontiguousarray(dec[b]),
                "w": w,
            }
        )
    return in_maps


def _run(encoder_outputs, decoder_outputs, W, trace=False):
    from concourse.bass_utils import run_bass_kernel_spmd

    nc = _get_nc()
    in_maps = _make_in_maps(encoder_outputs, decoder_outputs, W)
    res = run_bass_kernel_spmd(nc, in_maps, list(range(NCORES)), trace=trace)
    out = np.empty((B, T, U, C), dtype=np.float32)
    for i in range(NCORES):
        b, th = i // 2, i % 2
        out[b, th * TSH : (th + 1) * TSH] = res.results[i]["o"]
    return out, res


def kernel(encoder_outputs, decoder_outputs, W):
    out, _ = _run(encoder_outputs, decoder_outputs, W)
    return out
